# revision 1
# baseline (speedup 1.0000x reference)
"""Trainium2 Bass kernel for masked-softmax attention scoring.

Reference computation (B=128, T=512, K=1024, Q=1024):
    mids  = einsum("kq,bq->bk", W, query)
    s     = tanh(einsum("btk,bk->bt", key, mids) + bias)
    attn  = softmax-like: exp(s - max) * mask / sum(exp(s - max) * mask)

The max-subtraction cancels exactly in the ratio (tanh is bounded), so the
device computes  attn = exp(tanh(.)) * mask / sum_t(exp(tanh(.)) * mask).

Sharding: data-parallel over B across 8 NeuronCores (16 batches/core).
Per-core layout: partition p = (b, j) with b in [0,16), j in [0,8);
free column c in [0,64); timestep t = j*64 + c.

The mids matmul writes the (b, j)-replicated layout directly: the
stationary operand is query^T with each batch column replicated 8x via a
stride-0 DVE copy, fed as float32r (full-rate fp32 on the PE).  The W^T
prologue is split half-and-half across the two HWDGE FIFO rings so it
lands at aggregate HBM bandwidth; key chunks follow on both rings with
half-size chunks at the head and tail, consumed by 64 fused
multiply-reduce DVE ops (affine_mul_reduce) in merged arrival order.
Softmax normalization does the 8-partition group sum with a
block-diagonal 0/1 matmul.
"""

import sys

if "/opt/trn_rl_repo" not in sys.path:
    sys.path.insert(0, "/opt/trn_rl_repo")

from contextlib import ExitStack

import numpy as np

# ---- problem constants (hardcoded per spec) ----
B, T, K, Q = 128, 512, 1024, 1024
NCORES = 8
BS = B // NCORES          # 16 batches per core
P = 128                   # SBUF partitions
J = P // BS               # 8 t-blocks per batch on partitions
CF = T // J               # 64 timesteps per (partition, free col)
CC = 4                    # t-cols per key DMA super-chunk (2 MB each)
NCH = CF // CC            # 16 key DMAs per core
QC = Q // P               # 8 contraction chunks for the mids matmul
KEY_BUFS = 7              # key tile pool depth

_STATE: dict = {}


def _build_nc():
    import concourse.tile as tile
    from concourse import bacc, mybir

    f32 = mybir.dt.float32
    f32r = mybir.dt.float32r
    nc = bacc.Bacc()

    qt_e = nc.declare_dram_parameter("qt", [P, QC, BS], f32r, isOutput=False)
    wt_e = nc.declare_dram_parameter("wt", [P, QC, K], f32r, isOutput=False)
    grp_e = nc.declare_dram_parameter("grp", [P, P], f32, isOutput=False)
    key_e = nc.declare_dram_parameter("key", [BS, T, K], f32, isOutput=False)
    maskr_e = nc.declare_dram_parameter("maskr", [P, CF], f32, isOutput=False)
    bias_e = nc.declare_dram_parameter("biasb", [P, 1], f32, isOutput=False)
    out_e = nc.declare_dram_parameter("out", [P, CF], f32, isOutput=True)

    with tile.TileContext(nc) as tc, ExitStack() as ctx:
        const = ctx.enter_context(tc.tile_pool(name="const", bufs=1))
        kpool = ctx.enter_context(tc.tile_pool(name="key", bufs=KEY_BUFS))
        spool = ctx.enter_context(tc.tile_pool(name="scratch", bufs=2))
        psum = ctx.enter_context(tc.tile_pool(name="psum", bufs=1, space="PSUM"))

        # ---- prologue loads split across BOTH HWDGE rings (~2.1 MB each)
        # so W^T lands at full aggregate bandwidth (~20 us) and neither
        # ring idles before the key stream.
        qt_sb = const.tile([P, QC, BS], f32r)
        nc.sync.dma_start(out=qt_sb[:], in_=qt_e[:])
        wt_sb = const.tile([P, QC, K], f32r)
        for qc in range(QC // 2):
            nc.sync.dma_start(out=wt_sb[:, qc, :], in_=wt_e[:, qc, :])
        for qc in range(QC // 2, QC):
            nc.scalar.dma_start(out=wt_sb[:, qc, :], in_=wt_e[:, qc, :])
        grp_sb = const.tile([P, P], f32)
        nc.sync.dma_start(out=grp_sb[:], in_=grp_e[:])
        maskr_sb = const.tile([P, CF], f32)
        nc.sync.dma_start(out=maskr_sb[:], in_=maskr_e[:])
        bias_sb = const.tile([P, 1], f32)
        nc.sync.dma_start(out=bias_sb[:], in_=bias_e[:])

        # ---- mids in broadcast layout: [P, K], row p = mids[b(p), :] ----
        # Replicate each query column 8x on-chip (stride-0 DVE read) so the
        # stationary operand has the (b, j) partition order in one free dim.
        qtrep_sb = const.tile([P, QC, BS, J], f32r)
        nc.vector.tensor_copy(
            qtrep_sb[:], qt_sb[:].unsqueeze(-1).broadcast_to((P, QC, BS, J))
        )
        # matmuls in wt-chunk ARRIVAL order (rings deliver 0..3 and 4..7
        # concurrently); PSUM accumulation order is numerically immaterial.
        mids_ps = psum.tile([P, K], f32)
        qc_order = [0, 4, 1, 5, 2, 6, 3, 7]
        for qi, qc in enumerate(qc_order):
            lhsT = qtrep_sb[:, qc, :, :]
            for h in range(2):
                nc.tensor.matmul(
                    mids_ps[:, h * 512 : (h + 1) * 512],
                    lhsT=lhsT,
                    rhs=wt_sb[:, qc, h * 512 : (h + 1) * 512],
                    start=(qi == 0),
                    stop=(qi == QC - 1),
                )
        mids_bc = const.tile([P, K], f32)
        nc.vector.tensor_copy(mids_bc[:], mids_ps[:])

        # ---- scores[p, c] = key[b, j*64+c, :] . mids[b, :] ----
        # Both HWDGE FIFO rings stream 32 score-columns of key each, behind
        # their ~2.1 MB W^T halves; the final chunks are half-size so the
        # post-last-arrival DVE tail is short.  The DVE consumes chunks in
        # merged arrival order (model: equal per-ring column cadence).
        # Small chunks at the head (tolerate early arrival-order noise
        # cheaply) and at the tail (short post-last-arrival DVE tail).
        RING_COLS = {
            "A": [2, 2, 2, 2, 4, 4, 4, 4, 4, 2, 2],   # 32 cols
            "B": [4, 4, 4, 4, 4, 4, 4, 2, 2],          # 32 cols
        }
        entries = []
        for ring, pro in (("A", 6.8), ("B", 6.3)):
            t = pro
            for k, sz in enumerate(RING_COLS[ring]):
                t += sz * 2.9
                entries.append((t, ring, k, sz))
        entries.sort()
        scores_sb = const.tile([P, CF], f32)
        key_r = key_e[:].rearrange("b (j c) k -> (b j) c k", j=J)
        ring_eng = {"A": nc.sync, "B": nc.scalar}
        sched = []
        base = 0
        for t, ring, k, sz in entries:
            sched.append((ring, sz, base))
            base += sz
        for ring, sz, c0 in sched:
            kt = kpool.tile([P, CC, K], f32, tag="ktile")
            ring_eng[ring].dma_start(
                out=kt[:, 0:sz, :], in_=key_r[:, c0 : c0 + sz, :]
            )
            for cc in range(sz):
                c = c0 + cc
                prod = spool.tile([P, K], f32, tag="prod")
                nc.vector.affine_mul_reduce(
                    out=prod[:],
                    accum_out=scores_sb[:, c : c + 1],
                    in0=kt[:, cc, :],
                    in1=mids_bc[:],
                    scale=1.0,
                    bias=0.0,
                )

        # ---- epilogue: tanh, exp, mask, normalize ----
        tanh_sb = const.tile([P, CF], f32)
        nc.scalar.activation(
            out=tanh_sb[:],
            in_=scores_sb[:],
            func=mybir.ActivationFunctionType.Tanh,
            bias=bias_sb[:],
            scale=1.0,
        )
        exp_sb = const.tile([P, CF], f32)
        nc.scalar.activation(
            out=exp_sb[:], in_=tanh_sb[:], func=mybir.ActivationFunctionType.Exp
        )
        em_sb = const.tile([P, CF], f32)
        rowsum = const.tile([P, 1], f32)
        nc.vector.affine_mul_reduce(
            out=em_sb[:],
            accum_out=rowsum[:],
            in0=exp_sb[:],
            in1=maskr_sb[:],
            scale=1.0,
            bias=0.0,
        )
        den_ps = psum.tile([P, 1], f32)
        nc.tensor.matmul(
            den_ps[:], lhsT=grp_sb[:], rhs=rowsum[:], start=True, stop=True
        )
        rinv = const.tile([P, 1], f32)
        nc.vector.reciprocal(out=rinv[:], in_=den_ps[:])
        attn_sb = const.tile([P, CF], f32)
        nc.vector.tensor_scalar_mul(attn_sb[:], em_sb[:], rinv[:])
        nc.scalar.dma_start(out=out_e[:], in_=attn_sb[:])

    nc.compile()
    return nc


def _get_nc():
    if "nc" not in _STATE:
        _STATE["nc"] = _build_nc()
    return _STATE["nc"]


def _grp():
    if "GRP" not in _STATE:
        # GRP[p, m] = 1 iff p // J == m // J  (block-diagonal group-sum)
        pj = np.arange(P) // J
        _STATE["GRP"] = np.ascontiguousarray(
            (pj[:, None] == pj[None, :]).astype(np.float32)
        )
    return _STATE["GRP"]


def _make_in_maps(query, key, mask, W, bias):
    query = np.asarray(query, dtype=np.float32)
    key = np.asarray(key, dtype=np.float32)
    mask = np.asarray(mask, dtype=np.float32)
    W = np.asarray(W, dtype=np.float32)
    bias = np.asarray(bias, dtype=np.float32).reshape(-1)

    # wt[p, qc, k] = W.T[qc*128 + p, k]
    WT = np.ascontiguousarray(
        np.ascontiguousarray(W.T).reshape(QC, P, K).transpose(1, 0, 2)
    )
    GRP = _grp()
    biasb = np.ascontiguousarray(
        np.broadcast_to(bias[:1][None, :], (P, 1)).astype(np.float32)
    )

    in_maps = []
    for i in range(NCORES):
        sh = slice(i * BS, (i + 1) * BS)
        in_maps.append(
            {
                # pre-laid [P, QC, BS]: qt[p, qc, b] = query[sh].T[qc*128+p, b]
                "qt": np.ascontiguousarray(
                    query[sh].T.reshape(QC, P, BS).transpose(1, 0, 2)
                ),
                "wt": WT,
                "grp": GRP,
                "key": np.ascontiguousarray(key[sh]),
                "maskr": np.ascontiguousarray(mask[sh]).reshape(P, CF),
                "biasb": biasb,
            }
        )
    return in_maps


def _run(in_maps, **kwargs):
    from concourse.bass_utils import run_bass_kernel_spmd

    return run_bass_kernel_spmd(
        _get_nc(), in_maps, core_ids=list(range(NCORES)), **kwargs
    )


def _gather(results):
    return np.concatenate(
        [np.asarray(r["out"]).reshape(BS, T) for r in results], axis=0
    )


def kernel(query, key, mask, W, bias):
    in_maps = _make_in_maps(query, key, mask, W, bias)
    res = _run(in_maps)
    return _gather(res.results)



# revision 11
# speedup vs baseline: 1.3377x; 1.3377x over previous
"""Trainium2 Bass kernel for masked-softmax attention scoring (v2).

Reference computation (B=128, T=512, K=1024, Q=1024):
    mids  = einsum("kq,bq->bk", W, query)
    s     = tanh(einsum("btk,bk->bt", key, mids) + bias)
    attn  = softmax-like: exp(s - max) * mask / sum(exp(s - max) * mask)

The max-subtraction cancels exactly in the ratio (tanh is bounded), so the
device computes  attn = exp(tanh(.)) * mask / sum_t(exp(tanh(.)) * mask).

v2 design (vs v1's DVE mul-reduce):
- Everything 16-bit on the wire: key/W/query are cast to fp16 on the host
  (validated rel_l2 ~7e-4 vs the 2e-2 gate), halving the dominant HBM
  stream 36 MB -> 18 MB per core.
- The score dot-products move from the DVE (1.13 us per 1024-wide column,
  73 us/core total -- the v1 co-bottleneck) to the TensorEngine: the host
  pre-transposes key to [pair, kchunk, k-partition, (b0 t | b1 t)] so the
  PE contracts over k on partitions at ~0.42 ns/row (~28 us, hidden under
  DMA).  Stationary operand is midsT[:, kc, :] (all 16 batches); only the
  out row matching the rhs tile's batch is used; out tiles sit at
  32-aligned PSUM partition offsets (4 banks x 4 slots).
- mids^T is computed directly in [k-partition, batch] layout with the wt
  chunk as the stationary operand (64 small matmuls interleaved with W's
  arrival), avoiding any on-chip transpose.
- The otherwise-idle Scalar engine extracts each batch row from PSUM with
  a fused Tanh, then Exp, pipelined behind the PE stream.  The DVE only
  runs the tiny [16, Tc] mask/normalize epilogue.
- Optional mask compaction: masked-out timesteps (~14%. .20%) are never
  shipped; the host gathers kept t's per batch, the device computes only
  Tc = max kept count columns, and the host scatters rows back.

Engine queues: sync = DMA ring A; vector = DMA ring B + final epilogue;
gpsimd = midsT copy; scalar = per-batch tanh/exp; tensor = all matmuls.

Sharding: data-parallel over B across 8 NeuronCores (16 batches/core).
"""

import sys

if "/opt/trn_rl_repo" not in sys.path:
    sys.path.insert(0, "/opt/trn_rl_repo")

from contextlib import ExitStack

import numpy as np

# ---- problem constants (hardcoded per spec) ----
B, T, K, Q = 128, 512, 1024, 1024
NCORES = 8
BS = B // NCORES          # 16 batches per core
P = 128                   # SBUF partitions
QC = Q // P               # 8 contraction chunks for the mids matmul
KC = K // P               # 8 contraction chunks for the scores matmul
PR = BS // 2              # 8 batch pairs per core (2 batches per key tile)
KEY_BUFS = 16             # key tile pool depth (2 KB/partition each)
COMPACT = False           # gather kept timesteps on host, Tc = max count

_STATE: dict = {}


def _build_nc(Tc):
    import concourse.tile as tile
    from concourse import bacc, mybir

    f32 = mybir.dt.float32
    f16 = mybir.dt.float16
    nc = bacc.Bacc()

    qt_e = nc.declare_dram_parameter("qt", [P, QC, BS], f16, isOutput=False)
    wt_e = nc.declare_dram_parameter("wt", [P, QC, K], f16, isOutput=False)
    keyt_e = nc.declare_dram_parameter(
        "keyt", [PR, KC, P, 2 * Tc], f16, isOutput=False
    )
    maskb_e = nc.declare_dram_parameter("maskb", [BS, Tc], f32, isOutput=False)
    bias_e = nc.declare_dram_parameter("biasb", [P, 1], f32, isOutput=False)
    out_e = nc.declare_dram_parameter("out", [BS, Tc], f32, isOutput=True)

    with tile.TileContext(nc) as tc, ExitStack() as ctx:
        const = ctx.enter_context(tc.tile_pool(name="const", bufs=1))
        kpool = ctx.enter_context(tc.tile_pool(name="key", bufs=KEY_BUFS))
        psum = ctx.enter_context(tc.tile_pool(name="psum", bufs=1, space="PSUM"))

        # scores psum first so each [P, 512] tile is bank-aligned
        sc_ps = []
        for i in range(6):
            t = psum.tile([P, 512], f32, name=f"sc{i}")
            sc_ps.append(t)
        midsT_ps = psum.tile([P, KC, BS], f32)

        # ---- prologue loads split across both DMA rings ----
        qt_sb = const.tile([P, QC, BS], f16)
        nc.sync.dma_start(out=qt_sb[:], in_=qt_e[:])
        maskb_sb = const.tile([1, BS, Tc], f32)
        nc.scalar.dma_start(out=maskb_sb[:], in_=maskb_e[:].rearrange("(o b) t -> o b t", o=1))
        bias_sb = const.tile([P, 1], f32)
        nc.scalar.dma_start(out=bias_sb[:], in_=bias_e[:])
        wt_sb = const.tile([P, QC, K], f16)
        for qc in range(QC // 2):
            nc.sync.dma_start(out=wt_sb[:, qc, :], in_=wt_e[:, qc, :])
        for qc in range(QC // 2, QC):
            nc.scalar.dma_start(out=wt_sb[:, qc, :], in_=wt_e[:, qc, :])

        # ---- midsT[p, kc, b] = mids[b, kc*128+p], stationary = wt chunk ----
        # matmuls in wt-chunk ARRIVAL order (rings deliver 0..3 and 4..7
        # concurrently); PSUM accumulation order is numerically immaterial.
        # kc outer: exactly one PSUM accumulation group open at a time in
        # this bank -- interleaved groups at different byte offsets within
        # one bank accumulate incorrectly (scores groups at different
        # *partition* offsets are fine).
        qc_order = [0, 4, 1, 5, 2, 6, 3, 7]
        for kc in range(KC):
            for qi, qc in enumerate(qc_order):
                nc.tensor.matmul(
                    midsT_ps[:, kc, :],
                    lhsT=wt_sb[:, qc, kc * P : (kc + 1) * P],
                    rhs=qt_sb[:, qc, :],
                    start=(qi == 0),
                    stop=(qi == QC - 1),
                )
        midsT_sb = const.tile([P, KC, BS], f16)
        nc.vector.tensor_copy(midsT_sb[:], midsT_ps[:])

        # per-batch partition-0 tiles: every engine AP in the extraction/
        # epilogue chain starts at partition 0 or a 32-aligned psum base
        tanh_t = [const.tile([1, Tc], f32, name=f"tanh{b}") for b in range(BS)]
        exp_t = [const.tile([1, Tc], f32, name=f"exp{b}") for b in range(BS)]
        rsum_t = [const.tile([1, 1], f32, name=f"rsum{b}") for b in range(BS)]
        rinv_t = [const.tile([1, 1], f32, name=f"rinv{b}") for b in range(BS)]

        # ---- stream key tiles; PE accumulates scores over kc chunks ----
        # Stationary = one mids column (M=1), so batch b's score row lands
        # at its psum slice base partition, which must be 32-aligned for
        # both the PE tile_position and the Activation read (bank b//3,
        # offset 32*(b%3)).
        tile_order = []
        for i in range(4):
            for kc in range(KC):
                tile_order.append((nc.sync, i, kc))
                tile_order.append((nc.scalar, 4 + i, kc))
        for eng, pr, kc in tile_order:
            kt = kpool.tile([P, 2 * Tc], f16, tag="kt")
            eng.dma_start(out=kt[:], in_=keyt_e[pr, kc, :, :])
            for h in range(2):
                b = 2 * pr + h
                bank, slot = b // 3, b % 3
                nc.tensor.matmul(
                    sc_ps[bank][32 * slot : 32 * slot + 1, :Tc],
                    lhsT=midsT_sb[:, kc, b : b + 1],
                    rhs=kt[:, h * Tc : (h + 1) * Tc],
                    start=(kc == 0),
                    stop=(kc == KC - 1),
                )
            if kc == KC - 1:
                # pipelined per-batch epilogue: scalar does fused
                # extract+tanh from PSUM then exp; vector does mask+rowsum
                # (amr), reciprocal, scale; sync DMAs the finished row out.
                # Buffer reuse: masked-exp overwrites tanh_t, attn
                # overwrites exp_t.
                for h in range(2):
                    b = 2 * pr + h
                    bank, slot = b // 3, b % 3
                    row = 32 * slot
                    nc.scalar.activation(
                        out=tanh_t[b][:],
                        in_=sc_ps[bank][row : row + 1, :Tc],
                        func=mybir.ActivationFunctionType.Tanh,
                        bias=bias_sb[row : row + 1, :],
                        scale=1.0,
                    )
                    nc.scalar.activation(
                        out=exp_t[b][:],
                        in_=tanh_t[b][:],
                        func=mybir.ActivationFunctionType.Exp,
                    )
                    nc.vector.affine_mul_reduce(
                        out=tanh_t[b][:],
                        accum_out=rsum_t[b][:],
                        in0=exp_t[b][:],
                        in1=maskb_sb[:, b, :],
                        scale=1.0,
                        bias=0.0,
                    )
                    nc.vector.reciprocal(out=rinv_t[b][:], in_=rsum_t[b][:])
                    nc.vector.tensor_scalar_mul(
                        exp_t[b][:], tanh_t[b][:], rinv_t[b][:]
                    )
                    nc.sync.dma_start(out=out_e[b : b + 1, :], in_=exp_t[b][:])

    nc.compile()
    return nc


def _get_nc(Tc):
    if _STATE.get("Tc") != Tc:
        _STATE["nc"] = _build_nc(Tc)
        _STATE["Tc"] = Tc
    return _STATE["nc"]


def _make_in_maps(query, key, mask, W, bias):
    query = np.asarray(query, dtype=np.float32)
    key = np.asarray(key, dtype=np.float32)
    mask = np.asarray(mask, dtype=np.float32)
    W = np.asarray(W, dtype=np.float32)
    bias = np.asarray(bias, dtype=np.float32).reshape(-1)

    if COMPACT:
        kept = [np.flatnonzero(mask[b] > 0.5) for b in range(B)]
        ns = np.array([len(k) for k in kept])
        Tc = int(-(-ns.max() // 8) * 8)  # round up to multiple of 8
        if Tc > T:
            Tc = T
    else:
        kept = [np.arange(T) for _ in range(B)]
        ns = np.full(B, T)
        Tc = T
    _STATE["kept"] = kept
    _STATE["ns"] = ns
    _STATE["cur_Tc"] = Tc

    # wt[p, qc, k] = W[k, qc*128 + p]  (shared across cores)
    WT = np.ascontiguousarray(
        np.ascontiguousarray(W.T).reshape(QC, P, K).transpose(1, 0, 2)
    ).astype(np.float16)
    biasb = np.ascontiguousarray(
        np.broadcast_to(bias[:1][None, :], (P, 1)).astype(np.float32)
    )

    in_maps = []
    for i in range(NCORES):
        sh = slice(i * BS, (i + 1) * BS)
        qt = np.ascontiguousarray(
            query[sh].T.reshape(QC, P, BS).transpose(1, 0, 2)
        ).astype(np.float16)
        kk = key[sh]
        if COMPACT:
            kg = np.zeros((BS, Tc, K), dtype=np.float16)
            mb = np.zeros((BS, Tc), dtype=np.float32)
            for bb in range(BS):
                kb = kept[i * BS + bb]
                kg[bb, : len(kb)] = kk[bb, kb]
                mb[bb, : len(kb)] = 1.0
        else:
            kg = kk.astype(np.float16)
            mb = np.ascontiguousarray(mask[sh])
        # keyt[pr, kc, p, h*Tc + t] = kg[2*pr+h, t, kc*128+p]
        keyt = np.ascontiguousarray(
            kg.reshape(PR, 2, Tc, KC, P).transpose(0, 3, 4, 1, 2)
        ).reshape(PR, KC, P, 2 * Tc)
        in_maps.append(
            {
                "qt": qt,
                "wt": WT,
                "keyt": keyt,
                "maskb": mb,
                "biasb": biasb,
            }
        )
    return in_maps


def _run(in_maps, **kwargs):
    from concourse.bass_utils import run_bass_kernel_spmd

    return run_bass_kernel_spmd(
        _get_nc(_STATE["cur_Tc"]), in_maps, core_ids=list(range(NCORES)), **kwargs
    )


def _gather(results):
    out = np.zeros((B, T), dtype=np.float32)
    kept = _STATE["kept"]
    ns = _STATE["ns"]
    for i in range(NCORES):
        rows = np.asarray(results[i]["out"]).reshape(BS, _STATE["cur_Tc"])
        for bb in range(BS):
            b = i * BS + bb
            out[b, kept[b][: ns[b]]] = rows[bb, : ns[b]]
    return out


def kernel(query, key, mask, W, bias):
    in_maps = _make_in_maps(query, key, mask, W, bias)
    res = _run(in_maps)
    return _gather(res.results)


# revision 12
# speedup vs baseline: 1.5577x; 1.1645x over previous
"""Trainium2 Bass kernel for masked-softmax attention scoring (v3).

Reference computation (B=128, T=512, K=1024, Q=1024):
    mids  = einsum("kq,bq->bk", W, query)
    s     = tanh(einsum("btk,bk->bt", key, mids) + bias)
    attn  = softmax-like: exp(s - max) * mask / sum(exp(s - max) * mask)

The max-subtraction cancels exactly in the ratio (tanh is bounded), so the
device computes  attn = exp(tanh(.)) * mask / sum_t(exp(tanh(.)) * mask).

Design (evolved from the v1 DVE kernel via trace analysis):
- Everything 16-bit on the wire: key/W/query cast to fp16 on the host
  (rel_l2 ~1.4e-3 vs the 2e-2 gate), halving the dominant HBM stream.
- Score dot-products run on the TensorEngine (not the DVE, whose fused
  mul-reduce is 1.13 us per 1024-col column): the host pre-transposes key
  to [pair, kc-pair, k-partition, kcsub, (b0 t | b1 t)] so the PE
  contracts over k on partitions.  1 MB tiles (4 matmuls each) keep the
  per-dma_start sequencer cost (~630 ns) off the critical path; a single
  sync-queue ring fans out across all 16 hardware DMA queues.
- PSUM accumulation groups must each own a full bank (interleaved groups
  at different byte offsets within one bank accumulate incorrectly):
  mids^T uses 8 banks (kc -> bank kc, qc-outer so matmuls overlap W's
  arrival); scores then reuse 4 of those banks, one per in-flight batch
  (bank = 2*(pr%2) + h, WAR distance = one full pair).
- The otherwise-idle Scalar engine extracts each batch row from PSUM row 0
  with a fused Tanh, then Exp; the DVE does the per-batch mask+rowsum
  (affine_mul_reduce), reciprocal, and scale -- all pipelined behind the
  PE stream.  gpsimd (software DGE) issues the 16 tiny row DMAs out, so
  no hardware ring ever blocks on a compute dependency.
- Mask compaction (COMPACT=True): masked-out timesteps (~20%) are never
  shipped; the host gathers kept t's per batch, the device computes only
  Tc = max kept count columns, the host scatters rows back.  The NEFF is
  compiled for the actual Tc on first call.

Sharding: data-parallel over B across 8 NeuronCores (16 batches/core).
"""

import sys

if "/opt/trn_rl_repo" not in sys.path:
    sys.path.insert(0, "/opt/trn_rl_repo")

from contextlib import ExitStack

import numpy as np

# ---- problem constants (hardcoded per spec) ----
B, T, K, Q = 128, 512, 1024, 1024
NCORES = 8
BS = B // NCORES          # 16 batches per core
P = 128                   # SBUF partitions
QC = Q // P               # 8 contraction chunks for the mids matmul
KC = K // P               # 8 contraction chunks for the scores matmul
KCP = KC // 2             # kc pairs per key tile
PR = BS // 2              # 8 batch pairs per core (2 batches per key tile)
KEY_BUFS = 10             # key tile pool depth (4 KB/partition each)
COMPACT = False           # gather kept timesteps on host, Tc = max count

_STATE: dict = {}


def _build_nc(Tc):
    import concourse.tile as tile
    from concourse import bacc, mybir

    f32 = mybir.dt.float32
    f16 = mybir.dt.float16
    nc = bacc.Bacc()

    qt_e = nc.declare_dram_parameter("qt", [P, QC, BS], f16, isOutput=False)
    wt_e = nc.declare_dram_parameter("wt", [P, QC, K], f16, isOutput=False)
    keyt_e = nc.declare_dram_parameter(
        "keyt", [PR, KCP, P, 2, 2 * Tc], f16, isOutput=False
    )
    maskb_e = nc.declare_dram_parameter("maskb", [BS, Tc], f32, isOutput=False)
    bias_e = nc.declare_dram_parameter("biasb", [P, 1], f32, isOutput=False)
    out_e = nc.declare_dram_parameter("out", [BS, Tc], f32, isOutput=True)

    with tile.TileContext(nc) as tc, ExitStack() as ctx:
        const = ctx.enter_context(tc.tile_pool(name="const", bufs=1))
        kpool = ctx.enter_context(tc.tile_pool(name="key", bufs=KEY_BUFS))
        psum = ctx.enter_context(tc.tile_pool(name="psum", bufs=1, space="PSUM"))

        # 8 full psum banks: mids kc-group kc lives in pb[kc][:, :BS];
        # scores then reuse pb[0..3] row 0
        pb = [psum.tile([P, 512], f32, name=f"pb{i}") for i in range(8)]

        # ---- prologue loads ----
        qt_sb = const.tile([P, QC, BS], f16)
        nc.sync.dma_start(out=qt_sb[:], in_=qt_e[:])
        maskb_sb = const.tile([1, BS, Tc], f32)
        nc.scalar.dma_start(
            out=maskb_sb[:], in_=maskb_e[:].rearrange("(o b) t -> o b t", o=1)
        )
        bias_sb = const.tile([P, 1], f32)
        nc.scalar.dma_start(out=bias_sb[:], in_=bias_e[:])
        wt_sb = const.tile([P, QC, K], f16)
        for qc in range(QC // 2):
            nc.sync.dma_start(out=wt_sb[:, qc, :], in_=wt_e[:, qc, :])
        for qc in range(QC // 2, QC):
            nc.scalar.dma_start(out=wt_sb[:, qc, :], in_=wt_e[:, qc, :])

        # ---- midsT[p, kc, b] = mids[b, kc*128+p] ----
        # qc outer in ring-arrival order so matmuls overlap W's arrival;
        # each kc accumulation group owns its own psum bank.
        qc_order = [0, 4, 1, 5, 2, 6, 3, 7]
        for qi, qc in enumerate(qc_order):
            for kc in range(KC):
                nc.tensor.matmul(
                    pb[kc][:, :BS],
                    lhsT=wt_sb[:, qc, kc * P : (kc + 1) * P],
                    rhs=qt_sb[:, qc, :],
                    start=(qi == 0),
                    stop=(qi == QC - 1),
                )
        midsT_sb = const.tile([P, KC, BS], f16)
        for kc in range(KC):
            nc.vector.tensor_copy(midsT_sb[:, kc, :], pb[kc][:, :BS])

        # per-batch partition-0 tiles for the pipelined epilogue
        tanh_t = [const.tile([1, Tc], f32, name=f"tanh{b}") for b in range(BS)]
        exp_t = [const.tile([1, Tc], f32, name=f"exp{b}") for b in range(BS)]
        rsum_t = [const.tile([1, 1], f32, name=f"rsum{b}") for b in range(BS)]
        rinv_t = [const.tile([1, 1], f32, name=f"rinv{b}") for b in range(BS)]

        # ---- stream 1 MB key tiles on the sync ring; 4 matmuls per tile ----
        for pr in range(PR):
            for kcp in range(KCP):
                kt = kpool.tile([P, 2, 2 * Tc], f16, tag="kt")
                nc.sync.dma_start(out=kt[:], in_=keyt_e[pr, kcp, :, :, :])
                for j in range(2):
                    kc = 2 * kcp + j
                    for h in range(2):
                        b = 2 * pr + h
                        bank = 2 * (pr % 2) + h
                        nc.tensor.matmul(
                            pb[bank][0:1, :Tc],
                            lhsT=midsT_sb[:, kc, b : b + 1],
                            rhs=kt[:, j, h * Tc : (h + 1) * Tc],
                            start=(kc == 0),
                            stop=(kc == KC - 1),
                        )
            # pipelined per-batch epilogue: scalar does fused extract+tanh
            # from PSUM then exp; vector does mask+rowsum, reciprocal,
            # scale; gpsimd (software DGE) DMAs the finished row out.
            # Buffer reuse: masked-exp overwrites tanh_t, attn overwrites
            # exp_t.
            for h in range(2):
                b = 2 * pr + h
                bank = 2 * (pr % 2) + h
                nc.scalar.activation(
                    out=tanh_t[b][:],
                    in_=pb[bank][0:1, :Tc],
                    func=mybir.ActivationFunctionType.Tanh,
                    bias=bias_sb[0:1, :],
                    scale=1.0,
                )
                nc.scalar.activation(
                    out=exp_t[b][:],
                    in_=tanh_t[b][:],
                    func=mybir.ActivationFunctionType.Exp,
                )
                nc.vector.affine_mul_reduce(
                    out=tanh_t[b][:],
                    accum_out=rsum_t[b][:],
                    in0=exp_t[b][:],
                    in1=maskb_sb[:, b, :],
                    scale=1.0,
                    bias=0.0,
                )
                nc.vector.reciprocal(out=rinv_t[b][:], in_=rsum_t[b][:])
                nc.vector.tensor_scalar_mul(exp_t[b][:], tanh_t[b][:], rinv_t[b][:])
                nc.gpsimd.dma_start(out=out_e[b : b + 1, :], in_=exp_t[b][:])

    nc.compile()
    return nc


def _get_nc(Tc):
    if _STATE.get("Tc") != Tc:
        _STATE["nc"] = _build_nc(Tc)
        _STATE["Tc"] = Tc
    return _STATE["nc"]


def _make_in_maps(query, key, mask, W, bias):
    query = np.asarray(query, dtype=np.float32)
    key = np.asarray(key, dtype=np.float32)
    mask = np.asarray(mask, dtype=np.float32)
    W = np.asarray(W, dtype=np.float32)
    bias = np.asarray(bias, dtype=np.float32).reshape(-1)

    if COMPACT:
        kept = [np.flatnonzero(mask[b] > 0.5) for b in range(B)]
        ns = np.array([len(k) for k in kept])
        Tc = int(-(-ns.max() // 8) * 8)  # round up to multiple of 8
        if Tc > T:
            Tc = T
    else:
        kept = [np.arange(T) for _ in range(B)]
        ns = np.full(B, T)
        Tc = T
    _STATE["kept"] = kept
    _STATE["ns"] = ns
    _STATE["cur_Tc"] = Tc

    # wt[p, qc, k] = W[k, qc*128 + p]  (shared across cores)
    WT = np.ascontiguousarray(
        np.ascontiguousarray(W.T).reshape(QC, P, K).transpose(1, 0, 2)
    ).astype(np.float16)
    biasb = np.ascontiguousarray(
        np.broadcast_to(bias[:1][None, :], (P, 1)).astype(np.float32)
    )

    in_maps = []
    for i in range(NCORES):
        sh = slice(i * BS, (i + 1) * BS)
        qt = np.ascontiguousarray(
            query[sh].T.reshape(QC, P, BS).transpose(1, 0, 2)
        ).astype(np.float16)
        kk = key[sh]
        if COMPACT:
            kg = np.zeros((BS, Tc, K), dtype=np.float16)
            mb = np.zeros((BS, Tc), dtype=np.float32)
            for bb in range(BS):
                kb = kept[i * BS + bb]
                kg[bb, : len(kb)] = kk[bb, kb]
                mb[bb, : len(kb)] = 1.0
        else:
            kg = kk.astype(np.float16)
            mb = np.ascontiguousarray(mask[sh])
        # keyt[pr, kcp, p, j, h*Tc + t] = kg[2*pr+h, t, (2*kcp+j)*128+p]
        keyt = np.ascontiguousarray(
            kg.reshape(PR, 2, Tc, KCP, 2, P).transpose(0, 3, 5, 4, 1, 2)
        )
        in_maps.append(
            {
                "qt": qt,
                "wt": WT,
                "keyt": keyt,
                "maskb": mb,
                "biasb": biasb,
            }
        )
    return in_maps


def _run(in_maps, **kwargs):
    from concourse.bass_utils import run_bass_kernel_spmd

    return run_bass_kernel_spmd(
        _get_nc(_STATE["cur_Tc"]), in_maps, core_ids=list(range(NCORES)), **kwargs
    )


def _gather(results):
    out = np.zeros((B, T), dtype=np.float32)
    kept = _STATE["kept"]
    ns = _STATE["ns"]
    for i in range(NCORES):
        rows = np.asarray(results[i]["out"]).reshape(BS, _STATE["cur_Tc"])
        for bb in range(BS):
            b = i * BS + bb
            out[b, kept[b][: ns[b]]] = rows[bb, : ns[b]]
    return out


def kernel(query, key, mask, W, bias):
    in_maps = _make_in_maps(query, key, mask, W, bias)
    res = _run(in_maps)
    return _gather(res.results)


# revision 13
# speedup vs baseline: 1.6954x; 1.0884x over previous
"""Trainium2 Bass kernel for masked-softmax attention scoring (v3).

Reference computation (B=128, T=512, K=1024, Q=1024):
    mids  = einsum("kq,bq->bk", W, query)
    s     = tanh(einsum("btk,bk->bt", key, mids) + bias)
    attn  = softmax-like: exp(s - max) * mask / sum(exp(s - max) * mask)

The max-subtraction cancels exactly in the ratio (tanh is bounded), so the
device computes  attn = exp(tanh(.)) * mask / sum_t(exp(tanh(.)) * mask).

Design (evolved from the v1 DVE kernel via trace analysis):
- Everything 16-bit on the wire: key/W/query cast to fp16 on the host
  (rel_l2 ~1.4e-3 vs the 2e-2 gate), halving the dominant HBM stream.
- Score dot-products run on the TensorEngine (not the DVE, whose fused
  mul-reduce is 1.13 us per 1024-col column): the host pre-transposes key
  to [pair, kc-pair, k-partition, kcsub, (b0 t | b1 t)] so the PE
  contracts over k on partitions.  1 MB tiles (4 matmuls each) keep the
  per-dma_start sequencer cost (~630 ns) off the critical path; a single
  sync-queue ring fans out across all 16 hardware DMA queues.
- PSUM accumulation groups must each own a full bank (interleaved groups
  at different byte offsets within one bank accumulate incorrectly):
  mids^T uses 8 banks (kc -> bank kc, qc-outer so matmuls overlap W's
  arrival); scores then reuse 4 of those banks, one per in-flight batch
  (bank = 2*(pr%2) + h, WAR distance = one full pair).
- The otherwise-idle Scalar engine extracts each batch row from PSUM row 0
  with a fused Tanh, then Exp; the DVE does the per-batch mask+rowsum
  (affine_mul_reduce), reciprocal, and scale -- all pipelined behind the
  PE stream.  gpsimd (software DGE) issues the 16 tiny row DMAs out, so
  no hardware ring ever blocks on a compute dependency.
- Mask compaction (COMPACT=True): masked-out timesteps (~20%) are never
  shipped; the host gathers kept t's per batch, the device computes only
  Tc = max kept count columns, the host scatters rows back.  The NEFF is
  compiled for the actual Tc on first call.

Sharding: data-parallel over B across 8 NeuronCores (16 batches/core).
"""

import sys

if "/opt/trn_rl_repo" not in sys.path:
    sys.path.insert(0, "/opt/trn_rl_repo")

from contextlib import ExitStack

import numpy as np

# ---- problem constants (hardcoded per spec) ----
B, T, K, Q = 128, 512, 1024, 1024
NCORES = 8
BS = B // NCORES          # 16 batches per core
P = 128                   # SBUF partitions
QC = Q // P               # 8 contraction chunks for the mids matmul
KC = K // P               # 8 contraction chunks for the scores matmul
KCP = KC // 2             # kc pairs per key tile
PR = BS // 2              # 8 batch pairs per core (2 batches per key tile)
KEY_BUFS = 10             # key tile pool depth (4 KB/partition each)
COMPACT = False           # gather kept timesteps on host, Tc = max count

_STATE: dict = {}


def _build_nc(Tc):
    import concourse.tile as tile
    from concourse import bacc, mybir

    f32 = mybir.dt.float32
    f16 = mybir.dt.float16
    nc = bacc.Bacc()

    qt_e = nc.declare_dram_parameter("qt", [P, QC, BS], f16, isOutput=False)
    wt_e = nc.declare_dram_parameter("wt", [P, QC, K], f16, isOutput=False)
    keyt_e = nc.declare_dram_parameter(
        "keyt", [PR, KCP, P, 2, 2 * Tc], f16, isOutput=False
    )
    maskb_e = nc.declare_dram_parameter("maskb", [BS, Tc], f32, isOutput=False)
    bias_e = nc.declare_dram_parameter("biasb", [P, 1], f32, isOutput=False)
    out_e = nc.declare_dram_parameter("out", [BS, Tc], f32, isOutput=True)

    with tile.TileContext(nc) as tc, ExitStack() as ctx:
        const = ctx.enter_context(tc.tile_pool(name="const", bufs=1))
        kpool = ctx.enter_context(tc.tile_pool(name="key", bufs=KEY_BUFS))
        psum = ctx.enter_context(tc.tile_pool(name="psum", bufs=1, space="PSUM"))

        # 8 full psum banks: mids kc-group kc lives in pb[kc][:, :BS];
        # scores then reuse pb[0..3] row 0
        pb = [psum.tile([P, 512], f32, name=f"pb{i}") for i in range(8)]

        # ---- prologue loads ----
        qt_sb = const.tile([P, QC, BS], f16)
        nc.sync.dma_start(out=qt_sb[:], in_=qt_e[:])
        maskb_sb = const.tile([1, BS, Tc], f32)
        nc.scalar.dma_start(
            out=maskb_sb[:], in_=maskb_e[:].rearrange("(o b) t -> o b t", o=1)
        )
        bias_sb = const.tile([P, 1], f32)
        nc.scalar.dma_start(out=bias_sb[:], in_=bias_e[:])
        wt_sb = const.tile([P, QC, K], f16)
        nc.sync.dma_start(out=wt_sb[:, 0 : QC // 2, :], in_=wt_e[:, 0 : QC // 2, :])
        nc.scalar.dma_start(
            out=wt_sb[:, QC // 2 : QC, :], in_=wt_e[:, QC // 2 : QC, :]
        )

        # ---- midsT[p, kc, b] = mids[b, kc*128+p] ----
        # kc-outer groups (one open accumulation group per bank, banks 4-7
        # rotating), with each group's midsT slice copied right after its
        # stop.  Groups for kc pair X are emitted just before the score
        # matmuls that need them, so mids work fills PE gaps in the
        # DMA-paced stream instead of serializing ahead of it.
        midsT_sb = const.tile([P, KC, BS], f16)

        def mids_group(kc):
            bank = 4 + kc % 4
            for qc in range(QC):
                nc.tensor.matmul(
                    pb[bank][:, :BS],
                    lhsT=wt_sb[:, qc, kc * P : (kc + 1) * P],
                    rhs=qt_sb[:, qc, :],
                    start=(qc == 0),
                    stop=(qc == QC - 1),
                )
            nc.vector.tensor_copy(midsT_sb[:, kc, :], pb[bank][:, :BS])

        # per-batch partition-0 tiles for the pipelined epilogue
        tanh_t = [const.tile([1, Tc], f32, name=f"tanh{b}") for b in range(BS)]
        exp_t = [const.tile([1, Tc], f32, name=f"exp{b}") for b in range(BS)]
        rsum_t = [const.tile([1, 1], f32, name=f"rsum{b}") for b in range(BS)]
        rinv_t = [const.tile([1, 1], f32, name=f"rinv{b}") for b in range(BS)]

        mids_group(0)
        mids_group(1)

        # ---- stream 1 MB key tiles on the sync ring; 4 matmuls per tile ----
        for pr in range(PR):
            for kcp in range(KCP):
                if pr == 0 and kcp > 0:
                    mids_group(2 * kcp)
                    mids_group(2 * kcp + 1)
                kt = kpool.tile([P, 2, 2 * Tc], f16, tag="kt")
                nc.sync.dma_start(out=kt[:], in_=keyt_e[pr, kcp, :, :, :])
                for j in range(2):
                    kc = 2 * kcp + j
                    for h in range(2):
                        b = 2 * pr + h
                        bank = 2 * (pr % 2) + h
                        nc.tensor.matmul(
                            pb[bank][0:1, :Tc],
                            lhsT=midsT_sb[:, kc, b : b + 1],
                            rhs=kt[:, j, h * Tc : (h + 1) * Tc],
                            start=(kc == 0),
                            stop=(kc == KC - 1),
                        )
            # pipelined per-batch epilogue: scalar does fused extract+tanh
            # from PSUM then exp; vector does mask+rowsum, reciprocal,
            # scale; gpsimd (software DGE) DMAs the finished row out.
            # Buffer reuse: masked-exp overwrites tanh_t, attn overwrites
            # exp_t.
            for h in range(2):
                b = 2 * pr + h
                bank = 2 * (pr % 2) + h
                nc.scalar.activation(
                    out=tanh_t[b][:],
                    in_=pb[bank][0:1, :Tc],
                    func=mybir.ActivationFunctionType.Tanh,
                    bias=bias_sb[0:1, :],
                    scale=1.0,
                )
                nc.scalar.activation(
                    out=exp_t[b][:],
                    in_=tanh_t[b][:],
                    func=mybir.ActivationFunctionType.Exp,
                )
                nc.vector.affine_mul_reduce(
                    out=tanh_t[b][:],
                    accum_out=rsum_t[b][:],
                    in0=exp_t[b][:],
                    in1=maskb_sb[:, b, :],
                    scale=1.0,
                    bias=0.0,
                )
                nc.vector.reciprocal(out=rinv_t[b][:], in_=rsum_t[b][:])
                nc.vector.tensor_scalar_mul(exp_t[b][:], tanh_t[b][:], rinv_t[b][:])
                nc.gpsimd.dma_start(out=out_e[b : b + 1, :], in_=exp_t[b][:])

    nc.compile()
    return nc


def _get_nc(Tc):
    if _STATE.get("Tc") != Tc:
        _STATE["nc"] = _build_nc(Tc)
        _STATE["Tc"] = Tc
    return _STATE["nc"]


def _make_in_maps(query, key, mask, W, bias):
    query = np.asarray(query, dtype=np.float32)
    key = np.asarray(key, dtype=np.float32)
    mask = np.asarray(mask, dtype=np.float32)
    W = np.asarray(W, dtype=np.float32)
    bias = np.asarray(bias, dtype=np.float32).reshape(-1)

    if COMPACT:
        kept = [np.flatnonzero(mask[b] > 0.5) for b in range(B)]
        ns = np.array([len(k) for k in kept])
        Tc = int(-(-ns.max() // 8) * 8)  # round up to multiple of 8
        if Tc > T:
            Tc = T
    else:
        kept = [np.arange(T) for _ in range(B)]
        ns = np.full(B, T)
        Tc = T
    _STATE["kept"] = kept
    _STATE["ns"] = ns
    _STATE["cur_Tc"] = Tc

    # wt[p, qc, k] = W[k, qc*128 + p]  (shared across cores)
    WT = np.ascontiguousarray(
        np.ascontiguousarray(W.T).reshape(QC, P, K).transpose(1, 0, 2)
    ).astype(np.float16)
    biasb = np.ascontiguousarray(
        np.broadcast_to(bias[:1][None, :], (P, 1)).astype(np.float32)
    )

    in_maps = []
    for i in range(NCORES):
        sh = slice(i * BS, (i + 1) * BS)
        qt = np.ascontiguousarray(
            query[sh].T.reshape(QC, P, BS).transpose(1, 0, 2)
        ).astype(np.float16)
        kk = key[sh]
        if COMPACT:
            kg = np.zeros((BS, Tc, K), dtype=np.float16)
            mb = np.zeros((BS, Tc), dtype=np.float32)
            for bb in range(BS):
                kb = kept[i * BS + bb]
                kg[bb, : len(kb)] = kk[bb, kb]
                mb[bb, : len(kb)] = 1.0
        else:
            kg = kk.astype(np.float16)
            mb = np.ascontiguousarray(mask[sh])
        # keyt[pr, kcp, p, j, h*Tc + t] = kg[2*pr+h, t, (2*kcp+j)*128+p]
        keyt = np.ascontiguousarray(
            kg.reshape(PR, 2, Tc, KCP, 2, P).transpose(0, 3, 5, 4, 1, 2)
        )
        in_maps.append(
            {
                "qt": qt,
                "wt": WT,
                "keyt": keyt,
                "maskb": mb,
                "biasb": biasb,
            }
        )
    return in_maps


def _run(in_maps, **kwargs):
    from concourse.bass_utils import run_bass_kernel_spmd

    return run_bass_kernel_spmd(
        _get_nc(_STATE["cur_Tc"]), in_maps, core_ids=list(range(NCORES)), **kwargs
    )


def _gather(results):
    out = np.zeros((B, T), dtype=np.float32)
    kept = _STATE["kept"]
    ns = _STATE["ns"]
    for i in range(NCORES):
        rows = np.asarray(results[i]["out"]).reshape(BS, _STATE["cur_Tc"])
        for bb in range(BS):
            b = i * BS + bb
            out[b, kept[b][: ns[b]]] = rows[bb, : ns[b]]
    return out


def kernel(query, key, mask, W, bias):
    in_maps = _make_in_maps(query, key, mask, W, bias)
    res = _run(in_maps)
    return _gather(res.results)


# revision 14
# speedup vs baseline: 1.8312x; 1.0801x over previous
"""Trainium2 Bass kernel for masked-softmax attention scoring (v3).

Reference computation (B=128, T=512, K=1024, Q=1024):
    mids  = einsum("kq,bq->bk", W, query)
    s     = tanh(einsum("btk,bk->bt", key, mids) + bias)
    attn  = softmax-like: exp(s - max) * mask / sum(exp(s - max) * mask)

The max-subtraction cancels exactly in the ratio (tanh is bounded), so the
device computes  attn = exp(tanh(.)) * mask / sum_t(exp(tanh(.)) * mask).

Design (evolved from the v1 DVE kernel via trace analysis):
- Everything 16-bit on the wire: key/W/query cast to fp16 on the host
  (rel_l2 ~1.4e-3 vs the 2e-2 gate), halving the dominant HBM stream.
- Score dot-products run on the TensorEngine (not the DVE, whose fused
  mul-reduce is 1.13 us per 1024-col column): the host pre-transposes key
  to [pair, kc-pair, k-partition, kcsub, (b0 t | b1 t)] so the PE
  contracts over k on partitions.  1 MB tiles (4 matmuls each) keep the
  per-dma_start sequencer cost (~630 ns) off the critical path; a single
  sync-queue ring fans out across all 16 hardware DMA queues.
- PSUM accumulation groups must each own a full bank (interleaved groups
  at different byte offsets within one bank accumulate incorrectly):
  mids^T uses 8 banks (kc -> bank kc, qc-outer so matmuls overlap W's
  arrival); scores then reuse 4 of those banks, one per in-flight batch
  (bank = 2*(pr%2) + h, WAR distance = one full pair).
- The otherwise-idle Scalar engine extracts each batch row from PSUM row 0
  with a fused Tanh, then Exp; the DVE does the per-batch mask+rowsum
  (affine_mul_reduce), reciprocal, and scale -- all pipelined behind the
  PE stream.  gpsimd (software DGE) issues the 16 tiny row DMAs out, so
  no hardware ring ever blocks on a compute dependency.
- Mask compaction (COMPACT=True): masked-out timesteps (~20%) are never
  shipped; the host gathers kept t's per batch, the device computes only
  Tc = max kept count columns, the host scatters rows back.  The NEFF is
  compiled for the actual Tc on first call.

Sharding: data-parallel over B across 8 NeuronCores (16 batches/core).
"""

import sys

if "/opt/trn_rl_repo" not in sys.path:
    sys.path.insert(0, "/opt/trn_rl_repo")

from contextlib import ExitStack

import numpy as np

# ---- problem constants (hardcoded per spec) ----
B, T, K, Q = 128, 512, 1024, 1024
NCORES = 8
BS = B // NCORES          # 16 batches per core
P = 128                   # SBUF partitions
QC = Q // P               # 8 contraction chunks for the mids matmul
KC = K // P               # 8 contraction chunks for the scores matmul
KCP = KC // 2             # kc pairs per key tile
PR = BS // 2              # 8 batch pairs per core (2 batches per key tile)
KEY_BUFS = 10             # key tile pool depth (4 KB/partition each)
COMPACT = True           # gather kept timesteps on host, Tc = max count

_STATE: dict = {}


def _build_nc(Tc):
    import concourse.tile as tile
    from concourse import bacc, mybir

    f32 = mybir.dt.float32
    f16 = mybir.dt.float16
    nc = bacc.Bacc()

    qt_e = nc.declare_dram_parameter("qt", [P, QC, BS], f16, isOutput=False)
    wt_e = nc.declare_dram_parameter("wt", [P, QC, K], f16, isOutput=False)
    keyt_e = nc.declare_dram_parameter(
        "keyt", [PR, KCP, P, 2, 2 * Tc], f16, isOutput=False
    )
    maskb_e = nc.declare_dram_parameter("maskb", [BS, Tc], f32, isOutput=False)
    bias_e = nc.declare_dram_parameter("biasb", [P, 1], f32, isOutput=False)
    out_e = nc.declare_dram_parameter("out", [BS, Tc], f32, isOutput=True)

    with tile.TileContext(nc) as tc, ExitStack() as ctx:
        const = ctx.enter_context(tc.tile_pool(name="const", bufs=1))
        kpool = ctx.enter_context(tc.tile_pool(name="key", bufs=KEY_BUFS))
        psum = ctx.enter_context(tc.tile_pool(name="psum", bufs=1, space="PSUM"))

        # 8 full psum banks: mids kc-group kc lives in pb[kc][:, :BS];
        # scores then reuse pb[0..3] row 0
        pb = [psum.tile([P, 512], f32, name=f"pb{i}") for i in range(8)]

        # ---- prologue loads ----
        qt_sb = const.tile([P, QC, BS], f16)
        nc.sync.dma_start(out=qt_sb[:], in_=qt_e[:])
        maskb_sb = const.tile([1, BS, Tc], f32)
        nc.scalar.dma_start(
            out=maskb_sb[:], in_=maskb_e[:].rearrange("(o b) t -> o b t", o=1)
        )
        bias_sb = const.tile([P, 1], f32)
        nc.scalar.dma_start(out=bias_sb[:], in_=bias_e[:])
        wt_sb = const.tile([P, QC, K], f16)
        nc.sync.dma_start(out=wt_sb[:, 0 : QC // 2, :], in_=wt_e[:, 0 : QC // 2, :])
        nc.scalar.dma_start(
            out=wt_sb[:, QC // 2 : QC, :], in_=wt_e[:, QC // 2 : QC, :]
        )

        # ---- midsT[p, kc, b] = mids[b, kc*128+p] ----
        # kc-outer groups (one open accumulation group per bank, banks 4-7
        # rotating), with each group's midsT slice copied right after its
        # stop.  Groups for kc pair X are emitted just before the score
        # matmuls that need them, so mids work fills PE gaps in the
        # DMA-paced stream instead of serializing ahead of it.
        midsT_sb = const.tile([P, KC, BS], f16)

        def mids_group(kc):
            bank = 4 + kc % 4
            for qc in range(QC):
                nc.tensor.matmul(
                    pb[bank][:, :BS],
                    lhsT=wt_sb[:, qc, kc * P : (kc + 1) * P],
                    rhs=qt_sb[:, qc, :],
                    start=(qc == 0),
                    stop=(qc == QC - 1),
                )
            nc.vector.tensor_copy(midsT_sb[:, kc, :], pb[bank][:, :BS])

        # per-batch partition-0 tiles for the pipelined epilogue
        tanh_t = [const.tile([1, Tc], f32, name=f"tanh{b}") for b in range(BS)]
        exp_t = [const.tile([1, Tc], f32, name=f"exp{b}") for b in range(BS)]
        rsum_t = [const.tile([1, 1], f32, name=f"rsum{b}") for b in range(BS)]
        rinv_t = [const.tile([1, 1], f32, name=f"rinv{b}") for b in range(BS)]

        mids_group(0)
        mids_group(1)

        # ---- stream 1 MB key tiles on the sync ring; 4 matmuls per tile ----
        for pr in range(PR):
            for kcp in range(KCP):
                if pr == 0 and kcp > 0:
                    mids_group(2 * kcp)
                    mids_group(2 * kcp + 1)
                kt = kpool.tile([P, 2, 2 * Tc], f16, tag="kt")
                nc.sync.dma_start(out=kt[:], in_=keyt_e[pr, kcp, :, :, :])
                for j in range(2):
                    kc = 2 * kcp + j
                    for h in range(2):
                        b = 2 * pr + h
                        bank = 2 * (pr % 2) + h
                        nc.tensor.matmul(
                            pb[bank][0:1, :Tc],
                            lhsT=midsT_sb[:, kc, b : b + 1],
                            rhs=kt[:, j, h * Tc : (h + 1) * Tc],
                            start=(kc == 0),
                            stop=(kc == KC - 1),
                        )
            # pipelined per-batch epilogue: scalar does fused extract+tanh
            # from PSUM then exp; vector does mask+rowsum, reciprocal,
            # scale; gpsimd (software DGE) DMAs the finished row out.
            # Buffer reuse: masked-exp overwrites tanh_t, attn overwrites
            # exp_t.
            for h in range(2):
                b = 2 * pr + h
                bank = 2 * (pr % 2) + h
                nc.scalar.activation(
                    out=tanh_t[b][:],
                    in_=pb[bank][0:1, :Tc],
                    func=mybir.ActivationFunctionType.Tanh,
                    bias=bias_sb[0:1, :],
                    scale=1.0,
                )
                nc.scalar.activation(
                    out=exp_t[b][:],
                    in_=tanh_t[b][:],
                    func=mybir.ActivationFunctionType.Exp,
                )
                nc.vector.affine_mul_reduce(
                    out=tanh_t[b][:],
                    accum_out=rsum_t[b][:],
                    in0=exp_t[b][:],
                    in1=maskb_sb[:, b, :],
                    scale=1.0,
                    bias=0.0,
                )
                nc.vector.reciprocal(out=rinv_t[b][:], in_=rsum_t[b][:])
                nc.vector.tensor_scalar_mul(exp_t[b][:], tanh_t[b][:], rinv_t[b][:])
                nc.gpsimd.dma_start(out=out_e[b : b + 1, :], in_=exp_t[b][:])

    nc.compile()
    return nc


def _get_nc(Tc):
    if _STATE.get("Tc") != Tc:
        _STATE["nc"] = _build_nc(Tc)
        _STATE["Tc"] = Tc
    return _STATE["nc"]


def _make_in_maps(query, key, mask, W, bias):
    query = np.asarray(query, dtype=np.float32)
    key = np.asarray(key, dtype=np.float32)
    mask = np.asarray(mask, dtype=np.float32)
    W = np.asarray(W, dtype=np.float32)
    bias = np.asarray(bias, dtype=np.float32).reshape(-1)

    if COMPACT:
        kept = [np.flatnonzero(mask[b] > 0.5) for b in range(B)]
        ns = np.array([len(k) for k in kept])
        Tc = int(-(-ns.max() // 8) * 8)  # round up to multiple of 8
        if Tc > T:
            Tc = T
    else:
        kept = [np.arange(T) for _ in range(B)]
        ns = np.full(B, T)
        Tc = T
    _STATE["kept"] = kept
    _STATE["ns"] = ns
    _STATE["cur_Tc"] = Tc

    # wt[p, qc, k] = W[k, qc*128 + p]  (shared across cores)
    WT = np.ascontiguousarray(
        np.ascontiguousarray(W.T).reshape(QC, P, K).transpose(1, 0, 2)
    ).astype(np.float16)
    biasb = np.ascontiguousarray(
        np.broadcast_to(bias[:1][None, :], (P, 1)).astype(np.float32)
    )

    in_maps = []
    for i in range(NCORES):
        sh = slice(i * BS, (i + 1) * BS)
        qt = np.ascontiguousarray(
            query[sh].T.reshape(QC, P, BS).transpose(1, 0, 2)
        ).astype(np.float16)
        kk = key[sh]
        if COMPACT:
            kg = np.zeros((BS, Tc, K), dtype=np.float16)
            mb = np.zeros((BS, Tc), dtype=np.float32)
            for bb in range(BS):
                kb = kept[i * BS + bb]
                kg[bb, : len(kb)] = kk[bb, kb]
                mb[bb, : len(kb)] = 1.0
        else:
            kg = kk.astype(np.float16)
            mb = np.ascontiguousarray(mask[sh])
        # keyt[pr, kcp, p, j, h*Tc + t] = kg[2*pr+h, t, (2*kcp+j)*128+p]
        keyt = np.ascontiguousarray(
            kg.reshape(PR, 2, Tc, KCP, 2, P).transpose(0, 3, 5, 4, 1, 2)
        )
        in_maps.append(
            {
                "qt": qt,
                "wt": WT,
                "keyt": keyt,
                "maskb": mb,
                "biasb": biasb,
            }
        )
    return in_maps


def _run(in_maps, **kwargs):
    from concourse.bass_utils import run_bass_kernel_spmd

    return run_bass_kernel_spmd(
        _get_nc(_STATE["cur_Tc"]), in_maps, core_ids=list(range(NCORES)), **kwargs
    )


def _gather(results):
    out = np.zeros((B, T), dtype=np.float32)
    kept = _STATE["kept"]
    ns = _STATE["ns"]
    for i in range(NCORES):
        rows = np.asarray(results[i]["out"]).reshape(BS, _STATE["cur_Tc"])
        for bb in range(BS):
            b = i * BS + bb
            out[b, kept[b][: ns[b]]] = rows[bb, : ns[b]]
    return out


def kernel(query, key, mask, W, bias):
    in_maps = _make_in_maps(query, key, mask, W, bias)
    res = _run(in_maps)
    return _gather(res.results)


# revision 15
# speedup vs baseline: 1.8388x; 1.0041x over previous
"""Trainium2 Bass kernel for masked-softmax attention scoring (v3).

Reference computation (B=128, T=512, K=1024, Q=1024):
    mids  = einsum("kq,bq->bk", W, query)
    s     = tanh(einsum("btk,bk->bt", key, mids) + bias)
    attn  = softmax-like: exp(s - max) * mask / sum(exp(s - max) * mask)

The max-subtraction cancels exactly in the ratio (tanh is bounded), so the
device computes  attn = exp(tanh(.)) * mask / sum_t(exp(tanh(.)) * mask).

Design (evolved from the v1 DVE kernel via trace analysis):
- Everything 16-bit on the wire: key/W/query cast to fp16 on the host
  (rel_l2 ~1.4e-3 vs the 2e-2 gate), halving the dominant HBM stream.
- Score dot-products run on the TensorEngine (not the DVE, whose fused
  mul-reduce is 1.13 us per 1024-col column): the host pre-transposes key
  to [pair, kc-pair, k-partition, kcsub, (b0 t | b1 t)] so the PE
  contracts over k on partitions.  1 MB tiles (4 matmuls each) keep the
  per-dma_start sequencer cost (~630 ns) off the critical path; a single
  sync-queue ring fans out across all 16 hardware DMA queues.
- PSUM accumulation groups must each own a full bank (interleaved groups
  at different byte offsets within one bank accumulate incorrectly):
  mids^T uses 8 banks (kc -> bank kc, qc-outer so matmuls overlap W's
  arrival); scores then reuse 4 of those banks, one per in-flight batch
  (bank = 2*(pr%2) + h, WAR distance = one full pair).
- The otherwise-idle Scalar engine extracts each batch row from PSUM row 0
  with a fused Tanh, then Exp; the DVE does the per-batch mask+rowsum
  (affine_mul_reduce), reciprocal, and scale -- all pipelined behind the
  PE stream.  gpsimd (software DGE) issues the 16 tiny row DMAs out, so
  no hardware ring ever blocks on a compute dependency.
- Mask compaction (COMPACT=True): masked-out timesteps (~20%) are never
  shipped; the host gathers kept t's per batch, the device computes only
  Tc = max kept count columns, the host scatters rows back.  The NEFF is
  compiled for the actual Tc on first call.

Sharding: data-parallel over B across 8 NeuronCores (16 batches/core).
"""

import sys

if "/opt/trn_rl_repo" not in sys.path:
    sys.path.insert(0, "/opt/trn_rl_repo")

from contextlib import ExitStack

import numpy as np

# ---- problem constants (hardcoded per spec) ----
B, T, K, Q = 128, 512, 1024, 1024
NCORES = 8
BS = B // NCORES          # 16 batches per core
P = 128                   # SBUF partitions
QC = Q // P               # 8 contraction chunks for the mids matmul
KC = K // P               # 8 contraction chunks for the scores matmul
KCP = KC // 2             # kc pairs per key tile
PR = BS // 2              # 8 batch pairs per core (2 batches per key tile)
KEY_BUFS = 10             # key tile pool depth (4 KB/partition each)
COMPACT = True           # gather kept timesteps on host, Tc = max count

_STATE: dict = {}


def _build_nc(Tc):
    import concourse.tile as tile
    from concourse import bacc, mybir

    f32 = mybir.dt.float32
    f16 = mybir.dt.float16
    nc = bacc.Bacc()

    qt_e = nc.declare_dram_parameter("qt", [P, QC, BS], f16, isOutput=False)
    wt_e = nc.declare_dram_parameter("wt", [P, KC, QC, P], f16, isOutput=False)
    keyt_e = nc.declare_dram_parameter(
        "keyt", [PR, KCP, P, 2, 2 * Tc], f16, isOutput=False
    )
    maskb_e = nc.declare_dram_parameter("maskb", [BS, Tc], f32, isOutput=False)
    bias_e = nc.declare_dram_parameter("biasb", [P, 1], f32, isOutput=False)
    out_e = nc.declare_dram_parameter("out", [BS, Tc], f32, isOutput=True)

    with tile.TileContext(nc) as tc, ExitStack() as ctx:
        const = ctx.enter_context(tc.tile_pool(name="const", bufs=1))
        kpool = ctx.enter_context(tc.tile_pool(name="key", bufs=KEY_BUFS))
        psum = ctx.enter_context(tc.tile_pool(name="psum", bufs=1, space="PSUM"))

        # 8 full psum banks: mids kc-group kc lives in pb[kc][:, :BS];
        # scores then reuse pb[0..3] row 0
        pb = [psum.tile([P, 512], f32, name=f"pb{i}") for i in range(8)]

        # ---- prologue loads ----
        qt_sb = const.tile([P, QC, BS], f16)
        nc.sync.dma_start(out=qt_sb[:], in_=qt_e[:])
        # W streams kc-major: mids group kc only needs its own 256 KB
        # slice, so the first groups finish ~1.5 us after issue start
        # instead of waiting for all 2 MB of W.
        wt_sb = const.tile([P, KC, QC, P], f16)
        for kc in range(2):
            nc.sync.dma_start(out=wt_sb[:, kc, :, :], in_=wt_e[:, kc, :, :])
        for kc in range(2, KC):
            nc.scalar.dma_start(out=wt_sb[:, kc, :, :], in_=wt_e[:, kc, :, :])
        maskb_sb = const.tile([1, BS, Tc], f32)
        nc.scalar.dma_start(
            out=maskb_sb[:], in_=maskb_e[:].rearrange("(o b) t -> o b t", o=1)
        )
        bias_sb = const.tile([P, 1], f32)
        nc.scalar.dma_start(out=bias_sb[:], in_=bias_e[:])

        # ---- midsT[p, kc, b] = mids[b, kc*128+p] ----
        # kc-outer groups (one open accumulation group per bank, banks 4-7
        # rotating), with each group's midsT slice copied right after its
        # stop.  Groups for kc pair X are emitted just before the score
        # matmuls that need them, so mids work fills PE gaps in the
        # DMA-paced stream instead of serializing ahead of it.
        midsT_sb = const.tile([P, KC, BS], f16)

        def mids_group(kc):
            bank = 4 + kc % 4
            for qc in range(QC):
                nc.tensor.matmul(
                    pb[bank][:, :BS],
                    lhsT=wt_sb[:, kc, qc, :],
                    rhs=qt_sb[:, qc, :],
                    start=(qc == 0),
                    stop=(qc == QC - 1),
                )
            nc.vector.tensor_copy(midsT_sb[:, kc, :], pb[bank][:, :BS])

        # per-batch partition-0 tiles for the pipelined epilogue
        tanh_t = [const.tile([1, Tc], f32, name=f"tanh{b}") for b in range(BS)]
        exp_t = [const.tile([1, Tc], f32, name=f"exp{b}") for b in range(BS)]
        rsum_t = [const.tile([1, 1], f32, name=f"rsum{b}") for b in range(BS)]
        rinv_t = [const.tile([1, 1], f32, name=f"rinv{b}") for b in range(BS)]

        mids_group(0)
        mids_group(1)

        # ---- stream 1 MB key tiles on the sync ring; 4 matmuls per tile ----
        for pr in range(PR):
            for kcp in range(KCP):
                if pr == 0 and kcp > 0:
                    mids_group(2 * kcp)
                    mids_group(2 * kcp + 1)
                kt = kpool.tile([P, 2, 2 * Tc], f16, tag="kt")
                nc.sync.dma_start(out=kt[:], in_=keyt_e[pr, kcp, :, :, :])
                for j in range(2):
                    kc = 2 * kcp + j
                    for h in range(2):
                        b = 2 * pr + h
                        bank = 2 * (pr % 2) + h
                        nc.tensor.matmul(
                            pb[bank][0:1, :Tc],
                            lhsT=midsT_sb[:, kc, b : b + 1],
                            rhs=kt[:, j, h * Tc : (h + 1) * Tc],
                            start=(kc == 0),
                            stop=(kc == KC - 1),
                        )
            # pipelined per-batch epilogue: scalar does fused extract+tanh
            # from PSUM then exp; vector does mask+rowsum, reciprocal,
            # scale; gpsimd (software DGE) DMAs the finished row out.
            # Buffer reuse: masked-exp overwrites tanh_t, attn overwrites
            # exp_t.
            for h in range(2):
                b = 2 * pr + h
                bank = 2 * (pr % 2) + h
                nc.scalar.activation(
                    out=tanh_t[b][:],
                    in_=pb[bank][0:1, :Tc],
                    func=mybir.ActivationFunctionType.Tanh,
                    bias=bias_sb[0:1, :],
                    scale=1.0,
                )
                nc.scalar.activation(
                    out=exp_t[b][:],
                    in_=tanh_t[b][:],
                    func=mybir.ActivationFunctionType.Exp,
                )
                nc.vector.affine_mul_reduce(
                    out=tanh_t[b][:],
                    accum_out=rsum_t[b][:],
                    in0=exp_t[b][:],
                    in1=maskb_sb[:, b, :],
                    scale=1.0,
                    bias=0.0,
                )
                nc.vector.reciprocal(out=rinv_t[b][:], in_=rsum_t[b][:])
                nc.vector.tensor_scalar_mul(exp_t[b][:], tanh_t[b][:], rinv_t[b][:])
                nc.gpsimd.dma_start(out=out_e[b : b + 1, :], in_=exp_t[b][:])

    nc.compile()
    return nc


def _get_nc(Tc):
    if _STATE.get("Tc") != Tc:
        _STATE["nc"] = _build_nc(Tc)
        _STATE["Tc"] = Tc
    return _STATE["nc"]


def _make_in_maps(query, key, mask, W, bias):
    query = np.asarray(query, dtype=np.float32)
    key = np.asarray(key, dtype=np.float32)
    mask = np.asarray(mask, dtype=np.float32)
    W = np.asarray(W, dtype=np.float32)
    bias = np.asarray(bias, dtype=np.float32).reshape(-1)

    if COMPACT:
        kept = [np.flatnonzero(mask[b] > 0.5) for b in range(B)]
        ns = np.array([len(k) for k in kept])
        Tc = int(-(-ns.max() // 8) * 8)  # round up to multiple of 8
        if Tc > T:
            Tc = T
    else:
        kept = [np.arange(T) for _ in range(B)]
        ns = np.full(B, T)
        Tc = T
    _STATE["kept"] = kept
    _STATE["ns"] = ns
    _STATE["cur_Tc"] = Tc

    # wt[p, kc, qc, m] = W[kc*128+m, qc*128+p]  (kc-major; shared)
    WT = np.ascontiguousarray(
        np.ascontiguousarray(W.T).reshape(QC, P, KC, P).transpose(1, 2, 0, 3)
    ).astype(np.float16)
    biasb = np.ascontiguousarray(
        np.broadcast_to(bias[:1][None, :], (P, 1)).astype(np.float32)
    )

    in_maps = []
    for i in range(NCORES):
        sh = slice(i * BS, (i + 1) * BS)
        qt = np.ascontiguousarray(
            query[sh].T.reshape(QC, P, BS).transpose(1, 0, 2)
        ).astype(np.float16)
        kk = key[sh]
        if COMPACT:
            kg = np.zeros((BS, Tc, K), dtype=np.float16)
            mb = np.zeros((BS, Tc), dtype=np.float32)
            for bb in range(BS):
                kb = kept[i * BS + bb]
                kg[bb, : len(kb)] = kk[bb, kb]
                mb[bb, : len(kb)] = 1.0
        else:
            kg = kk.astype(np.float16)
            mb = np.ascontiguousarray(mask[sh])
        # keyt[pr, kcp, p, j, h*Tc + t] = kg[2*pr+h, t, (2*kcp+j)*128+p]
        keyt = np.ascontiguousarray(
            kg.reshape(PR, 2, Tc, KCP, 2, P).transpose(0, 3, 5, 4, 1, 2)
        )
        in_maps.append(
            {
                "qt": qt,
                "wt": WT,
                "keyt": keyt,
                "maskb": mb,
                "biasb": biasb,
            }
        )
    return in_maps


def _run(in_maps, **kwargs):
    from concourse.bass_utils import run_bass_kernel_spmd

    return run_bass_kernel_spmd(
        _get_nc(_STATE["cur_Tc"]), in_maps, core_ids=list(range(NCORES)), **kwargs
    )


def _gather(results):
    out = np.zeros((B, T), dtype=np.float32)
    kept = _STATE["kept"]
    ns = _STATE["ns"]
    for i in range(NCORES):
        rows = np.asarray(results[i]["out"]).reshape(BS, _STATE["cur_Tc"])
        for bb in range(BS):
            b = i * BS + bb
            out[b, kept[b][: ns[b]]] = rows[bb, : ns[b]]
    return out


def kernel(query, key, mask, W, bias):
    in_maps = _make_in_maps(query, key, mask, W, bias)
    res = _run(in_maps)
    return _gather(res.results)
